# revision 16
# baseline (speedup 1.0000x reference)
"""Trainium2 Bass kernel for a sigmoid-scored attention decode step with KV cache.

Reference computation (all fp32):
    q = W_query @ x.T ; k = W_key @ x.T ; v = W_value @ x.T          # [4096, 1]
    K = [K_cache | k] ; V = [V_cache | v]                            # [4096, 8193]
    a = sigmoid((q.T @ K) / 64)                                      # [1, 8193]
    z = V @ a.T                                                      # [4096, 1]

Sharding: rows (output dim) of W_q/W_k/W_v/K_cache/V_cache are split across
8 NeuronCores (512 rows each). Each core computes its q/k/v shard and partial
scores over its 512 rows of K; per-chunk AllReduces (4x ~8KB) combine partials
into full scores on every core; sigmoid + the V-weighted sum are then local
per shard. Host only slices inputs and concatenates the output.

Engine mapping per core:
  - q/k/v matvecs and z = V@a contract along the free dim -> DVE custom-op
    TENSOR_TENSOR_REDUCE against a broadcast vector (the native ISA
    tensor_tensor_reduce faults the exec unit on this runtime).
  - scores q.T K contract along partitions -> PE matmuls, K_cache tiles in
    natural [d, t] layout.
  - the score vector is AllReduced in 4 column chunks so collectives,
    sigmoid, PE rank-1 broadcast (ones x a_chunk -> PSUM) and the z-phase
    DVE reduces pipeline against the V_cache DMA stream instead of
    serializing at the end.
"""

import sys

for _p in ("/opt/trn_rl_repo", "/root/.axon_site/_ro/trn_rl_repo"):
    if _p not in sys.path:
        sys.path.append(_p)

import numpy as np

import concourse.bacc as bacc
import concourse.tile as tile
from concourse import mybir
from concourse.bass_utils import run_bass_kernel_spmd
from concourse.dve_ops import TENSOR_TENSOR_REDUCE

N_CORES = 8
E = 4096          # embedding dim (contraction for q/k/v)
D = 4096          # output dim
T = 8192          # cached timesteps
F32 = mybir.dt.float32


def build(n_cores=N_CORES, e=E, d_sh=D // N_CORES, t=T, kv_f=4096, w_f=4096):
    nd = d_sh // 128             # partition-chunks per core
    nc_t = t // kv_f             # cache column groups (2 at kv_f=4096)
    nj = kv_f // 512             # matmul slices per cache tile
    bps_f = min(2048, kv_f)      # broadcast-PSUM chunk (<=4 banks)
    nh = kv_f // bps_f

    nc = bacc.Bacc("TRN2", target_bir_lowering=False, debug=False,
                   num_devices=n_cores)
    x_d = nc.dram_tensor("x", [1, e], F32, kind="ExternalInput").ap()
    wq_d = nc.dram_tensor("wq", [d_sh, e], F32, kind="ExternalInput").ap()
    wk_d = nc.dram_tensor("wk", [d_sh, e], F32, kind="ExternalInput").ap()
    wv_d = nc.dram_tensor("wv", [d_sh, e], F32, kind="ExternalInput").ap()
    kc_d = nc.dram_tensor("kc", [d_sh, t], F32, kind="ExternalInput").ap()
    vc_d = nc.dram_tensor("vc", [d_sh, t], F32, kind="ExternalInput").ap()
    z_d = nc.dram_tensor("z", [128, nd], F32, kind="ExternalOutput").ap()

    with tile.TileContext(nc) as tc:
        with (
            tc.tile_pool(name="w", bufs=3) as wp,            # x + W tiles
            tc.tile_pool(name="stream", bufs=4) as sp,       # K/V cache tiles
            tc.tile_pool(name="scratch", bufs=1) as scp,     # ttr elementwise outs
            tc.tile_pool(name="keep", bufs=1) as kp,         # persistent tiles
            tc.tile_pool(name="acc", bufs=8) as accp,        # [128,1] accumulators
            tc.tile_pool(name="dram", bufs=1, space="DRAM") as dramp,
        ):
            # --- broadcast x across partitions ---
            x_sb = wp.tile([1, e], F32, tag="w", name="x_sb")
            nc.gpsimd.dma_start(x_sb[:], x_d[:])
            bx = kp.tile([128, e], F32, tag="bx", name="bx")
            nc.gpsimd.partition_broadcast(bx[:], x_sb[:])

            ones_sb = kp.tile([1, 128], F32, tag="ones", name="ones_sb")
            nc.vector.memset(ones_sb[:], 1.0)
            ones_col = kp.tile([128, 1], F32, tag="onesc", name="ones_col")
            nc.vector.memset(ones_col[:], 1.0)
            # pre-warm the sigmoid ACT table so the load is off the critical path
            warm = kp.tile([1, 1], F32, tag="warm", name="warm")
            nc.vector.memset(warm[:], 0.0)
            nc.scalar.activation(warm[:], warm[:],
                                 mybir.ActivationFunctionType.Sigmoid,
                                 scale=1.0 / 64.0)

            # --- q/k/v matvecs: qkv_all[:, nd*w + d] = (W[d-chunk] @ x) ---
            qkv_all = kp.tile([128, 3 * nd], F32, tag="qkv", name="qkv_all")

            def w_matvec(w_dram, col0):
                for d in range(nd):
                    wt = wp.tile([128, w_f], F32, tag="w", name=f"wt{col0}_{d}")
                    nc.sync.dma_start(wt[:], w_dram[128 * d:128 * (d + 1), :])
                    sc = scp.tile([128, w_f], F32, tag="sc", name=f"sc{col0}_{d}")
                    nc.vector._custom_dve(
                        TENSOR_TENSOR_REDUCE, out=sc[:], in0=wt[:], in1=bx[:],
                        s0=0.0, s1=1.0,
                        accum_out=qkv_all[:, col0 + d:col0 + d + 1],
                    )

            w_matvec(wq_d, 0)        # q in cols 0..nd-1

            # --- partial scores per column group; AR_0 fires after group 0 ---
            s_sb = kp.tile([1, t + 8], F32, tag="s", name="s_sb")
            a_sb = s_sb  # AR results land back in the same buffer, chunk-local
            nc.vector.memset(s_sb[0:1, t:t + 8], 0.0)
            g0_len = kv_f
            cc_ins = [dramp.tile([1, g0_len], F32, tag="cc_in0", name="cc_in0"),
                      dramp.tile([1, t - g0_len], F32, tag="cc_in1",
                                 name="cc_in1"),
                      dramp.tile([1, 8], F32, tag="cc_in2", name="cc_in2")]
            cc_outs = [dramp.tile([1, g0_len], F32, tag="cc_out0", name="cc_out0"),
                       dramp.tile([1, t - g0_len], F32, tag="cc_out1",
                                  name="cc_out1"),
                       dramp.tile([1, 8], F32, tag="cc_out2", name="cc_out2")]

            psp_ctx = tc.tile_pool(name="ps", bufs=8, space="PSUM")
            psp = psp_ctx.__enter__()

            def score_group(c):
                pss = [psp.tile([1, 512], F32, tag="ps", name=f"ps{c}_{j}")
                       for j in range(nj)]
                for d in range(nd):
                    kt = sp.tile([128, kv_f], F32, tag="kv", name=f"kt{c}_{d}")
                    nc.sync.dma_start(
                        kt[:], kc_d[128 * d:128 * (d + 1),
                                    kv_f * c:kv_f * (c + 1)])
                    for j in range(nj):
                        nc.tensor.matmul(
                            pss[j][:],
                            lhsT=qkv_all[:, d:d + 1],
                            rhs=kt[:, 512 * j:512 * (j + 1)],
                            start=(d == 0), stop=(d == nd - 1),
                        )
                for j in range(nj):
                    nc.vector.tensor_copy(
                        s_sb[0:1, kv_f * c + 512 * j:kv_f * c + 512 * (j + 1)],
                        pss[j][:])

            score_group(0)
            nc.scalar.dma_start(cc_ins[0][:], s_sb[0:1, 0:g0_len])
            nc.gpsimd.collective_compute(
                "AllReduce", mybir.AluOpType.add,
                replica_groups=[list(range(n_cores))],
                ins=[cc_ins[0].opt()], outs=[cc_outs[0].opt()],
            )
            w_matvec(wk_d, nd)       # k in cols nd..2nd-1
            for c in range(1, nc_t):
                score_group(c)

            nc.scalar.dma_start(cc_ins[1][:], s_sb[0:1, g0_len:t])
            nc.gpsimd.collective_compute(
                "AllReduce", mybir.AluOpType.add,
                replica_groups=[list(range(n_cores))],
                ins=[cc_ins[1].opt()], outs=[cc_outs[1].opt()],
            )

            # --- appended-column score rides its own tiny AR ---
            qk_el = scp.tile([128, nd], F32, tag="qk_el", name="qk_el")
            qk_part = accp.tile([128, 1], F32, tag="acc", name="qk_part")
            nc.vector._custom_dve(
                TENSOR_TENSOR_REDUCE, out=qk_el[:], in0=qkv_all[:, 0:nd],
                in1=qkv_all[:, nd:2 * nd], s0=0.0, s1=1.0,
                accum_out=qk_part[:],
            )
            qk_ps = psp.tile([1, 512], F32, tag="ps", name="qk_ps")
            nc.tensor.matmul(qk_ps[0:1, 0:1], lhsT=ones_col[:],
                             rhs=qk_part[:], start=True, stop=True)
            nc.vector.tensor_copy(s_sb[0:1, t:t + 1], qk_ps[0:1, 0:1])
            nc.gpsimd.dma_start(cc_ins[2][:], s_sb[0:1, t:t + 8])
            nc.gpsimd.collective_compute(
                "AllReduce", mybir.AluOpType.add,
                replica_groups=[list(range(n_cores))],
                ins=[cc_ins[2].opt()], outs=[cc_outs[2].opt()],
            )
            w_matvec(wv_d, 2 * nd)   # v in cols 2nd..3nd-1
            psp_ctx.__exit__(None, None, None)

            # --- per 2048-subchunk: sigmoid -> PE rank-1 broadcast into PSUM;
            # --- z accumulation: DVE reduce of V tiles against broadcast a ---
            z_final = kp.tile([128, nd], F32, tag="z", name="z_final")
            with tc.tile_pool(name="bps", bufs=2, space="PSUM") as bpsp:
                accs = [None] * nd
                for c in range(nc_t):
                    if c == 0:
                        nc.scalar.dma_start(a_sb[0:1, 0:g0_len], cc_outs[0][:])
                    else:
                        nc.scalar.dma_start(a_sb[0:1, g0_len:t],
                                            cc_outs[1][:])
                    bps_tiles = []
                    for h in range(nh):
                        sub = kv_f * c + bps_f * h
                        clen = bps_f
                        nc.scalar.activation(a_sb[0:1, sub:sub + clen],
                                             a_sb[0:1, sub:sub + clen],
                                             mybir.ActivationFunctionType.Sigmoid,
                                             scale=1.0 / 64.0)
                        bps = bpsp.tile([128, bps_f], F32, tag="bps",
                                        name=f"bps{c}_{h}")
                        for j in range(bps_f // 512):
                            nc.tensor.matmul(
                                bps[:, 512 * j:512 * (j + 1)],
                                lhsT=ones_sb[:],
                                rhs=a_sb[0:1, sub + 512 * j:sub + 512 * (j + 1)],
                                start=True, stop=True,
                            )
                        bps_tiles.append(bps)
                    for d in range(nd):
                        vt = sp.tile([128, kv_f], F32, tag="kv", name=f"vt{c}_{d}")
                        nc.sync.dma_start(
                            vt[:], vc_d[128 * d:128 * (d + 1),
                                        kv_f * c:kv_f * (c + 1)])
                        for h in range(nh):
                            sc = scp.tile([128, bps_f], F32, tag="zsc",
                                          name=f"zs{c}_{h}_{d}")
                            acc = accp.tile([128, 1], F32, tag="acc",
                                            name=f"za{c}_{h}_{d}")
                            nc.vector._custom_dve(
                                TENSOR_TENSOR_REDUCE, out=sc[:],
                                in0=vt[:, bps_f * h:bps_f * (h + 1)],
                                in1=bps_tiles[h][:],
                                s0=0.0 if accs[d] is None else accs[d][:],
                                s1=1.0,
                                accum_out=acc[:],
                            )
                            accs[d] = acc

                # --- final column: z += v * a[t] ---
                nc.scalar.dma_start(a_sb[0:1, t:t + 8], cc_outs[2][:])
                nc.scalar.activation(a_sb[0:1, t:t + 1], a_sb[0:1, t:t + 1],
                                     mybir.ActivationFunctionType.Sigmoid,
                                     scale=1.0 / 64.0)
                a_last_b = kp.tile([128, 1], F32, tag="alb", name="a_last_b")
                nc.gpsimd.partition_broadcast(a_last_b[:], a_sb[0:1, t:t + 1])
                for d in range(nd):
                    sc1 = scp.tile([128, 1], F32, tag="sc1", name=f"zf{d}")
                    nc.vector._custom_dve(
                        TENSOR_TENSOR_REDUCE, out=sc1[:],
                        in0=qkv_all[:, 2 * nd + d:2 * nd + d + 1],
                        in1=a_last_b[:],
                        s0=accs[d][:], s1=1.0,
                        accum_out=z_final[:, d:d + 1],
                    )

                nc.gpsimd.dma_start(z_d[:], z_final[:])

    nc.compile()
    return nc


def make_in_maps(inputs, n_cores=N_CORES, d_sh=D // N_CORES):
    x = np.ascontiguousarray(np.asarray(inputs["x"], dtype=np.float32))
    in_maps = []
    for i in range(n_cores):
        r0, r1 = d_sh * i, d_sh * (i + 1)
        in_maps.append({
            "x": x,
            "wq": np.ascontiguousarray(np.asarray(inputs["W_query"])[r0:r1], np.float32),
            "wk": np.ascontiguousarray(np.asarray(inputs["W_key"])[r0:r1], np.float32),
            "wv": np.ascontiguousarray(np.asarray(inputs["W_value"])[r0:r1], np.float32),
            "kc": np.ascontiguousarray(np.asarray(inputs["K_cache"])[r0:r1], np.float32),
            "vc": np.ascontiguousarray(np.asarray(inputs["V_cache"])[r0:r1], np.float32),
        })
    return in_maps


def unshard(per_core_z, d_sh=D // N_CORES):
    shards = [np.asarray(zi).T.reshape(d_sh, 1) for zi in per_core_z]
    return np.concatenate(shards, axis=0).astype(np.float32)


_NC_CACHE = None


def kernel(x, W_query, W_key, W_value, K_cache, V_cache):
    global _NC_CACHE
    if _NC_CACHE is None:
        _NC_CACHE = build()
    nc = _NC_CACHE
    in_maps = make_in_maps(dict(x=x, W_query=W_query, W_key=W_key,
                                W_value=W_value, K_cache=K_cache,
                                V_cache=V_cache))
    res = run_bass_kernel_spmd(nc, in_maps, core_ids=list(range(N_CORES)))
    return unshard([res.results[i]["z"] for i in range(N_CORES)])


# revision 17
# speedup vs baseline: 1.0074x; 1.0074x over previous
"""Trainium2 Bass kernel for a sigmoid-scored attention decode step with KV cache.

Reference computation (all fp32):
    q = W_query @ x.T ; k = W_key @ x.T ; v = W_value @ x.T          # [4096, 1]
    K = [K_cache | k] ; V = [V_cache | v]                            # [4096, 8193]
    a = sigmoid((q.T @ K) / 64)                                      # [1, 8193]
    z = V @ a.T                                                      # [4096, 1]

Sharding: rows (output dim) of W_q/W_k/W_v/K_cache/V_cache are split across
8 NeuronCores (512 rows each). Each core computes its q/k/v shard and partial
scores over its 512 rows of K; per-chunk AllReduces (4x ~8KB) combine partials
into full scores on every core; sigmoid + the V-weighted sum are then local
per shard. Host only slices inputs and concatenates the output.

Engine mapping per core:
  - q/k/v matvecs and z = V@a contract along the free dim -> DVE custom-op
    TENSOR_TENSOR_REDUCE against a broadcast vector (the native ISA
    tensor_tensor_reduce faults the exec unit on this runtime).
  - scores q.T K contract along partitions -> PE matmuls, K_cache tiles in
    natural [d, t] layout.
  - the score vector is AllReduced in 4 column chunks so collectives,
    sigmoid, PE rank-1 broadcast (ones x a_chunk -> PSUM) and the z-phase
    DVE reduces pipeline against the V_cache DMA stream instead of
    serializing at the end.
"""

import sys

for _p in ("/opt/trn_rl_repo", "/root/.axon_site/_ro/trn_rl_repo"):
    if _p not in sys.path:
        sys.path.append(_p)

import numpy as np

import concourse.bacc as bacc
import concourse.tile as tile
from concourse import mybir
from concourse.bass_utils import run_bass_kernel_spmd
from concourse.dve_ops import TENSOR_TENSOR_REDUCE

N_CORES = 8
E = 4096          # embedding dim (contraction for q/k/v)
D = 4096          # output dim
T = 8192          # cached timesteps
F32 = mybir.dt.float32


def build(n_cores=N_CORES, e=E, d_sh=D // N_CORES, t=T, kv_f=4096, w_f=4096):
    nd = d_sh // 128             # partition-chunks per core
    nc_t = t // kv_f             # cache column groups (2 at kv_f=4096)
    nj = kv_f // 512             # matmul slices per cache tile
    bps_f = min(2048, kv_f)      # broadcast-PSUM chunk (<=4 banks)
    nh = kv_f // bps_f

    nc = bacc.Bacc("TRN2", target_bir_lowering=False, debug=False,
                   num_devices=n_cores)
    x_d = nc.dram_tensor("x", [1, e], F32, kind="ExternalInput").ap()
    wq_d = nc.dram_tensor("wq", [d_sh, e], F32, kind="ExternalInput").ap()
    wk_d = nc.dram_tensor("wk", [d_sh, e], F32, kind="ExternalInput").ap()
    wv_d = nc.dram_tensor("wv", [d_sh, e], F32, kind="ExternalInput").ap()
    kc_d = nc.dram_tensor("kc", [d_sh, t], F32, kind="ExternalInput").ap()
    vc_d = nc.dram_tensor("vc", [d_sh, t], F32, kind="ExternalInput").ap()
    z_d = nc.dram_tensor("z", [128, nd], F32, kind="ExternalOutput").ap()

    with tile.TileContext(nc) as tc:
        with (
            tc.tile_pool(name="w", bufs=3) as wp,            # x + W tiles
            tc.tile_pool(name="stream", bufs=3) as sp,       # K/V cache tiles
            tc.tile_pool(name="scratch", bufs=2) as scp,     # ttr elementwise outs
            tc.tile_pool(name="keep", bufs=1) as kp,         # persistent tiles
            tc.tile_pool(name="acc", bufs=8) as accp,        # [128,1] accumulators
            tc.tile_pool(name="dram", bufs=1, space="DRAM") as dramp,
        ):
            # --- broadcast x across partitions ---
            x_sb = wp.tile([1, e], F32, tag="w", name="x_sb")
            nc.gpsimd.dma_start(x_sb[:], x_d[:])
            bx = kp.tile([128, e], F32, tag="bx", name="bx")
            nc.gpsimd.partition_broadcast(bx[:], x_sb[:])

            ones_sb = kp.tile([1, 128], F32, tag="ones", name="ones_sb")
            nc.vector.memset(ones_sb[:], 1.0)
            ones_col = kp.tile([128, 1], F32, tag="onesc", name="ones_col")
            nc.vector.memset(ones_col[:], 1.0)
            # pre-warm the sigmoid ACT table so the load is off the critical path
            warm = kp.tile([1, 1], F32, tag="warm", name="warm")
            nc.vector.memset(warm[:], 0.0)
            nc.scalar.activation(warm[:], warm[:],
                                 mybir.ActivationFunctionType.Sigmoid,
                                 scale=1.0 / 64.0)

            # --- q/k/v matvecs: qkv_all[:, nd*w + d] = (W[d-chunk] @ x) ---
            qkv_all = kp.tile([128, 3 * nd], F32, tag="qkv", name="qkv_all")

            def w_matvec(w_dram, col0):
                for d in range(nd):
                    wt = wp.tile([128, w_f], F32, tag="w", name=f"wt{col0}_{d}")
                    nc.sync.dma_start(wt[:], w_dram[128 * d:128 * (d + 1), :])
                    sc = scp.tile([128, w_f], F32, tag="sc", name=f"sc{col0}_{d}")
                    nc.vector._custom_dve(
                        TENSOR_TENSOR_REDUCE, out=sc[:], in0=wt[:], in1=bx[:],
                        s0=0.0, s1=1.0,
                        accum_out=qkv_all[:, col0 + d:col0 + d + 1],
                    )

            w_matvec(wq_d, 0)        # q in cols 0..nd-1

            # --- partial scores per column group; AR_0 fires after group 0 ---
            s_sb = kp.tile([1, t + 8], F32, tag="s", name="s_sb")
            a_sb = s_sb  # AR results land back in the same buffer, chunk-local
            nc.vector.memset(s_sb[0:1, t:t + 8], 0.0)
            g0_len = kv_f
            cc_ins = [dramp.tile([1, g0_len], F32, tag="cc_in0", name="cc_in0"),
                      dramp.tile([1, t - g0_len], F32, tag="cc_in1",
                                 name="cc_in1"),
                      dramp.tile([1, 8], F32, tag="cc_in2", name="cc_in2")]
            cc_outs = [dramp.tile([1, g0_len], F32, tag="cc_out0", name="cc_out0"),
                       dramp.tile([1, t - g0_len], F32, tag="cc_out1",
                                  name="cc_out1"),
                       dramp.tile([1, 8], F32, tag="cc_out2", name="cc_out2")]

            psp_ctx = tc.tile_pool(name="ps", bufs=8, space="PSUM")
            psp = psp_ctx.__enter__()

            def score_group(c):
                pss = [psp.tile([1, 512], F32, tag="ps", name=f"ps{c}_{j}")
                       for j in range(nj)]
                for d in range(nd):
                    kt = sp.tile([128, kv_f], F32, tag="kv", name=f"kt{c}_{d}")
                    nc.sync.dma_start(
                        kt[:], kc_d[128 * d:128 * (d + 1),
                                    kv_f * c:kv_f * (c + 1)])
                    for j in range(nj):
                        nc.tensor.matmul(
                            pss[j][:],
                            lhsT=qkv_all[:, d:d + 1],
                            rhs=kt[:, 512 * j:512 * (j + 1)],
                            start=(d == 0), stop=(d == nd - 1),
                        )
                for j in range(nj):
                    nc.vector.tensor_copy(
                        s_sb[0:1, kv_f * c + 512 * j:kv_f * c + 512 * (j + 1)],
                        pss[j][:])

            score_group(0)
            nc.gpsimd.dma_start(cc_ins[0][:], s_sb[0:1, 0:g0_len])
            nc.gpsimd.collective_compute(
                "AllReduce", mybir.AluOpType.add,
                replica_groups=[list(range(n_cores))],
                ins=[cc_ins[0].opt()], outs=[cc_outs[0].opt()],
            )
            w_matvec(wk_d, nd)       # k in cols nd..2nd-1
            for c in range(1, nc_t):
                score_group(c)

            nc.gpsimd.dma_start(cc_ins[1][:], s_sb[0:1, g0_len:t])
            nc.gpsimd.collective_compute(
                "AllReduce", mybir.AluOpType.add,
                replica_groups=[list(range(n_cores))],
                ins=[cc_ins[1].opt()], outs=[cc_outs[1].opt()],
            )

            # --- appended-column score rides its own tiny AR ---
            qk_el = scp.tile([128, nd], F32, tag="qk_el", name="qk_el")
            qk_part = accp.tile([128, 1], F32, tag="acc", name="qk_part")
            nc.vector._custom_dve(
                TENSOR_TENSOR_REDUCE, out=qk_el[:], in0=qkv_all[:, 0:nd],
                in1=qkv_all[:, nd:2 * nd], s0=0.0, s1=1.0,
                accum_out=qk_part[:],
            )
            qk_ps = psp.tile([1, 512], F32, tag="ps", name="qk_ps")
            nc.tensor.matmul(qk_ps[0:1, 0:1], lhsT=ones_col[:],
                             rhs=qk_part[:], start=True, stop=True)
            nc.vector.tensor_copy(s_sb[0:1, t:t + 1], qk_ps[0:1, 0:1])
            nc.gpsimd.dma_start(cc_ins[2][:], s_sb[0:1, t:t + 8])
            nc.gpsimd.collective_compute(
                "AllReduce", mybir.AluOpType.add,
                replica_groups=[list(range(n_cores))],
                ins=[cc_ins[2].opt()], outs=[cc_outs[2].opt()],
            )
            w_matvec(wv_d, 2 * nd)   # v in cols 2nd..3nd-1
            psp_ctx.__exit__(None, None, None)

            # --- per 2048-subchunk: sigmoid -> PE rank-1 broadcast into PSUM;
            # --- z accumulation: DVE reduce of V tiles against broadcast a ---
            z_final = kp.tile([128, nd], F32, tag="z", name="z_final")
            with tc.tile_pool(name="bps", bufs=2, space="PSUM") as bpsp:
                accs = [None] * nd
                for c in range(nc_t):
                    if c == 0:
                        nc.scalar.dma_start(a_sb[0:1, 0:g0_len], cc_outs[0][:])
                    else:
                        nc.scalar.dma_start(a_sb[0:1, g0_len:t],
                                            cc_outs[1][:])
                    bps_tiles = []
                    for h in range(nh):
                        sub = kv_f * c + bps_f * h
                        clen = bps_f
                        nc.scalar.activation(a_sb[0:1, sub:sub + clen],
                                             a_sb[0:1, sub:sub + clen],
                                             mybir.ActivationFunctionType.Sigmoid,
                                             scale=1.0 / 64.0)
                        bps = bpsp.tile([128, bps_f], F32, tag="bps",
                                        name=f"bps{c}_{h}")
                        for j in range(bps_f // 512):
                            nc.tensor.matmul(
                                bps[:, 512 * j:512 * (j + 1)],
                                lhsT=ones_sb[:],
                                rhs=a_sb[0:1, sub + 512 * j:sub + 512 * (j + 1)],
                                start=True, stop=True,
                            )
                        bps_tiles.append(bps)
                    for d in range(nd):
                        vt = sp.tile([128, kv_f], F32, tag="kv", name=f"vt{c}_{d}")
                        nc.sync.dma_start(
                            vt[:], vc_d[128 * d:128 * (d + 1),
                                        kv_f * c:kv_f * (c + 1)])
                        for h in range(nh):
                            sc = scp.tile([128, bps_f], F32, tag="zsc",
                                          name=f"zs{c}_{h}_{d}")
                            acc = accp.tile([128, 1], F32, tag="acc",
                                            name=f"za{c}_{h}_{d}")
                            nc.vector._custom_dve(
                                TENSOR_TENSOR_REDUCE, out=sc[:],
                                in0=vt[:, bps_f * h:bps_f * (h + 1)],
                                in1=bps_tiles[h][:],
                                s0=0.0 if accs[d] is None else accs[d][:],
                                s1=1.0,
                                accum_out=acc[:],
                            )
                            accs[d] = acc

                # --- final column: z += v * a[t] ---
                nc.scalar.dma_start(a_sb[0:1, t:t + 8], cc_outs[2][:])
                nc.scalar.activation(a_sb[0:1, t:t + 1], a_sb[0:1, t:t + 1],
                                     mybir.ActivationFunctionType.Sigmoid,
                                     scale=1.0 / 64.0)
                a_last_b = kp.tile([128, 1], F32, tag="alb", name="a_last_b")
                nc.gpsimd.partition_broadcast(a_last_b[:], a_sb[0:1, t:t + 1])
                for d in range(nd):
                    sc1 = scp.tile([128, 1], F32, tag="sc1", name=f"zf{d}")
                    nc.vector._custom_dve(
                        TENSOR_TENSOR_REDUCE, out=sc1[:],
                        in0=qkv_all[:, 2 * nd + d:2 * nd + d + 1],
                        in1=a_last_b[:],
                        s0=accs[d][:], s1=1.0,
                        accum_out=z_final[:, d:d + 1],
                    )

                nc.gpsimd.dma_start(z_d[:], z_final[:])

    nc.compile()
    return nc


def make_in_maps(inputs, n_cores=N_CORES, d_sh=D // N_CORES):
    x = np.ascontiguousarray(np.asarray(inputs["x"], dtype=np.float32))
    in_maps = []
    for i in range(n_cores):
        r0, r1 = d_sh * i, d_sh * (i + 1)
        in_maps.append({
            "x": x,
            "wq": np.ascontiguousarray(np.asarray(inputs["W_query"])[r0:r1], np.float32),
            "wk": np.ascontiguousarray(np.asarray(inputs["W_key"])[r0:r1], np.float32),
            "wv": np.ascontiguousarray(np.asarray(inputs["W_value"])[r0:r1], np.float32),
            "kc": np.ascontiguousarray(np.asarray(inputs["K_cache"])[r0:r1], np.float32),
            "vc": np.ascontiguousarray(np.asarray(inputs["V_cache"])[r0:r1], np.float32),
        })
    return in_maps


def unshard(per_core_z, d_sh=D // N_CORES):
    shards = [np.asarray(zi).T.reshape(d_sh, 1) for zi in per_core_z]
    return np.concatenate(shards, axis=0).astype(np.float32)


_NC_CACHE = None


def kernel(x, W_query, W_key, W_value, K_cache, V_cache):
    global _NC_CACHE
    if _NC_CACHE is None:
        _NC_CACHE = build()
    nc = _NC_CACHE
    in_maps = make_in_maps(dict(x=x, W_query=W_query, W_key=W_key,
                                W_value=W_value, K_cache=K_cache,
                                V_cache=V_cache))
    res = run_bass_kernel_spmd(nc, in_maps, core_ids=list(range(N_CORES)))
    return unshard([res.results[i]["z"] for i in range(N_CORES)])
